# revision 25
# baseline (speedup 1.0000x reference)
"""AnchorAttention distributed Bass kernel for 8 TRN2 NeuronCores.

Reference computation (B=2, S=4096, D=1024, H=16, Dh=64, A=512):
  anchors = x[:, :A];  queries = x[:, A:]
  anchor_q/k/v = split_heads(anchors @ Wq/Wk/Wv + b)
  query_q      = split_heads(queries @ Wqt + bqt)
  combined_q   = concat([anchor_q, query_q], axis=2)       # [B,H,S,Dh]
  out  = softmax(combined_q @ anchor_k^T / sqrt(Dh)) @ anchor_v
  out  = merge_heads(out) @ Wo + bo

Sharding: the B*S = 8192 token rows are split into 8 chunks of 1024 rows
(core c -> batch c//4, rows (c%4)*1024 ...). Each core duplicates its
batch's anchor K/V projections, computes Q for its own rows (Wq for the
anchor-region rows, Wqt for query rows), attention over the 512 anchors
for all 16 heads, and the output projection for its rows. The output is a
pure concatenation: no collectives.

Layout: everything is kept transposed ([feature, row]) so each matmul
contracts over the partition dim with zero on-chip transposes. Host
pre-transposes/pre-casts to bf16 (f32 accumulation in PSUM). Weights are
laid out head-group(ct)-major so each group's projection only depends on
a small prefix of the DMA stream; K/V projections are woven per-group
into the attention pipeline so compute starts as soon as the first
chunks land. A short burst of dummy matmuls at t=0 warms the PE clock
gate (HAM) before real work arrives.

Scores contract 64-deep per head, so each (row-chunk, anchor-chunk)
score pair runs as two concurrent PE row-tiles ((0,0) even head /
(64,0) odd head) writing separate PSUM banks -- 2x score throughput and
no zero-padded Q slabs. Score spans are emitted in blocks of two so the
PE only pays a row-mode transition at block boundaries.

Softmax row-sums come free via an extra all-ones column appended to V
(no max-subtraction is needed; scores are ~N(0,1)). 1/sums: sums rows
are copied from PSUM to fixed parity SBUF tiles (custom-DVE reciprocal
cannot read PSUM), reciprocal'd + cast on Vector, partition-broadcast by
a [65,128] 0/1-selector matmul, and multiplied into the attn^T slab.

Output projection: bo is folded into the PE accumulation via a
ones-first-row stationary matmul against a broadcast bo tile, so the
PSUM evacuation is a plain copy, split between the Scalar and Vector
engines, cast to bf16, and DMA'd out per 128-row block on alternating
queues so the drain pipelines with the O-proj matmul stream.
"""

import numpy as np
import ml_dtypes

import concourse.bass as bass
import concourse.tile as tile
from concourse import bacc, mybir
from concourse import bass_utils

BF16 = mybir.dt.bfloat16
F32 = mybir.dt.float32
B, S, D = 2, 4096, 1024
H, DH = 16, 64
A = 512                  # num_anchor_tokens (asserted at runtime)
RPC = 1024               # rows per core
NCORES = 8
SCALE = 1.0 / np.sqrt(float(DH))

_CACHE = {}


def _build():
    """Build + compile the per-core Bass graph (identical on all cores)."""
    nc = bacc.Bacc("TRN2", target_bir_lowering=False, debug=False)

    # [feat%128, rc, dt, row]: row chunks of x^T, dt = feat//128
    xt = nc.dram_tensor("xt", [128, 2, 8, 512], BF16, kind="ExternalInput")
    at = nc.dram_tensor("at", [128, 8, A], BF16, kind="ExternalInput")
    # Q weights, group-major: [in%128, ct, rc, dt, out-slice]
    wq2 = nc.dram_tensor("wq2", [128, 8, 2, 8, 128], BF16,
                         kind="ExternalInput")
    wk = nc.dram_tensor("wk", [128, 8, 8, 128], BF16, kind="ExternalInput")
    wv = nc.dram_tensor("wv", [128, 2, 8, 512], BF16, kind="ExternalInput")
    wo = nc.dram_tensor("wo", [128, 8, D], BF16, kind="ExternalInput")
    bq2 = nc.dram_tensor("bq2", [128, 2, 8], F32, kind="ExternalInput")
    bk = nc.dram_tensor("bk", [128, 8], F32, kind="ExternalInput")
    bv = nc.dram_tensor("bv", [128, D], BF16, kind="ExternalInput")  # bcast
    bo = nc.dram_tensor("bo", [128, D], BF16, kind="ExternalInput")  # bcast
    out = nc.dram_tensor("out", [RPC, D], BF16, kind="ExternalOutput")

    Exp = mybir.ActivationFunctionType.Exp
    Ident = mybir.ActivationFunctionType.Identity

    from contextlib import ExitStack

    with tile.TileContext(nc) as tc:
        with tc.tile_pool(name="wpool", bufs=1) as wpool, \
             tc.tile_pool(name="cpool", bufs=1) as cpool, \
             tc.tile_pool(name="kvpool", bufs=1) as kvpool, \
             tc.tile_pool(name="ptpool", bufs=4) as ptpool, \
             tc.tile_pool(name="tmppool", bufs=2) as tmppool, \
             tc.tile_pool(name="outpool", bufs=2) as outpool, \
             tc.tile_pool(name="attnpool", bufs=1) as attnpool, \
             tc.tile_pool(name="psum", bufs=2, space="PSUM") as psum:


            # ---- constants (GpSimd so the DMA queues stay clear) ----
            sel_sb = cpool.tile([65, 128], BF16, name="sel_sb")
            nc.gpsimd.memset(sel_sb, 0.0)
            nc.gpsimd.memset(sel_sb[0:1, 0:64], 1.0)
            nc.gpsimd.memset(sel_sb[64:65, 64:128], 1.0)
            # ones-first-row stationary for the O-proj bias accumulation
            ones1 = cpool.tile([128, 128], BF16, name="ones1")
            nc.gpsimd.memset(ones1, 0.0)
            nc.gpsimd.memset(ones1[0:1, :], 1.0)
            wtile = cpool.tile([65, 128], BF16, name="wtile")
            nc.gpsimd.memset(wtile, 0.0)

            # ---- input DMAs, deadline-ordered, chunked across the three
            # HWDGE dispatch queues (sync / scalar / gpsimd). ----
            at_sb = wpool.tile([128, 8, A], BF16, name="at_sb")
            wk_sb = wpool.tile([128, 8, 8, 128], BF16, name="wk_sb")
            xt_sb = wpool.tile([128, 2, 8, 512], BF16, name="xt_sb")
            wv_sb = wpool.tile([128, 2, 8, 512], BF16, name="wv_sb")
            wq2_sb = wpool.tile([128, 8, 2, 8, 128], BF16, name="wq2_sb")
            wo_sb = wpool.tile([128, 8, D], BF16, name="wo_sb")

            nc.sync.dma_start(out=at_sb[:, 0:4, :], in_=at.ap()[:, 0:4, :])
            nc.sync.dma_start(out=wk_sb[:, 0:2], in_=wk.ap()[:, 0:2])
            nc.sync.dma_start(out=xt_sb[:, 0], in_=xt.ap()[:, 0])
            nc.sync.dma_start(out=wk_sb[:, 2:4], in_=wk.ap()[:, 2:4])
            nc.sync.dma_start(out=wv_sb[:, 0], in_=wv.ap()[:, 0])
            nc.sync.dma_start(out=wk_sb[:, 4:8], in_=wk.ap()[:, 4:8])
            nc.sync.dma_start(out=wo_sb[:, 0:4, :], in_=wo.ap()[:, 0:4, :])
            nc.sync.dma_start(out=wo_sb[:, 4:8, :], in_=wo.ap()[:, 4:8, :])

            nc.scalar.dma_start(out=at_sb[:, 4:8, :], in_=at.ap()[:, 4:8, :])
            nc.scalar.dma_start(out=wq2_sb[:, 0:2], in_=wq2.ap()[:, 0:2])
            nc.scalar.dma_start(out=xt_sb[:, 1], in_=xt.ap()[:, 1])
            nc.scalar.dma_start(out=wq2_sb[:, 2:4], in_=wq2.ap()[:, 2:4])


            bq2_sb = cpool.tile([128, 2, 8], F32, name="bq2_sb")
            bk_sb = cpool.tile([128, 8], F32, name="bk_sb")
            bv_bc = cpool.tile([128, D], BF16, name="bv_bc")
            bo_bc = cpool.tile([128, D], BF16, name="bo_bc")
            nc.gpsimd.dma_start(out=bq2_sb, in_=bq2.ap())
            nc.gpsimd.dma_start(out=bk_sb, in_=bk.ap())
            nc.gpsimd.dma_start(out=bv_bc, in_=bv.ap())
            nc.gpsimd.dma_start(out=bo_bc, in_=bo.ap())
            nc.gpsimd.dma_start(out=wq2_sb[:, 4:6], in_=wq2.ap()[:, 4:6])
            nc.gpsimd.dma_start(out=wv_sb[:, 1], in_=wv.ap()[:, 1])
            nc.gpsimd.dma_start(out=wq2_sb[:, 6:8], in_=wq2.ap()[:, 6:8])

            # ---- PE warmup: dep-free dummy matmuls fill the DMA head so
            # the HAM clock gate is at full rate when real work arrives ----
            pwarm = psum.tile([128, 128], F32, tag="s", name="pwarm", bufs=2)
            for _ in range(96):
                nc.tensor.matmul(pwarm, sel_sb, wtile, start=True, stop=True)

            # fixed parity tiles for the 1/sums chain; rows other than 0/64
            # hold 1.0 forever so the reciprocal/cast stay finite
            sums4s, rcpbfs = [], []
            rcp4 = cpool.tile([65, 2, 512], F32, name="rcp4")
            for eo in range(2):
                s4p = cpool.tile([65, 2, 512], F32, name=f"sums4_{eo}")
                nc.vector.memset(s4p, 1.0)
                sums4s.append(s4p)
                rcpbfs.append(cpool.tile([65, 2, 512], BF16,
                                         name=f"rcpbf_{eo}"))

            # V slab: [128(a%128), ach, head, 65]; cols 0-63 = V head slice,
            # col 64 = ones (supplies softmax row-sums during AV).
            vaug = kvpool.tile([128, 4, H, DH + 1], BF16, name="vaug")
            nc.vector.memset(vaug, 1.0)

            kts, qts = {}, {}
            attnT = attnpool.tile([128, 8, RPC], BF16, name="attnT")

            # ---- per-group projections (woven into the pipeline) ----
            def kproj(ct):
                kts[ct] = kvpool.tile([128, A], BF16, tag="kt",
                                      name="kt", bufs=3)
                pk = psum.tile([128, A], F32, tag="work", name="pk")
                for dt in range(8):
                    nc.tensor.matmul(
                        pk, wk_sb[:, ct, dt, :], at_sb[:, dt, :],
                        start=(dt == 0), stop=(dt == 7))
                nc.scalar.activation(out=kts[ct], in_=pk,
                                     func=Ident, bias=bk_sb[:, ct:ct + 1])

            def qproj_rc(ct, rc):
                if rc == 0:
                    qts[ct] = kvpool.tile([128, 2, 512], BF16, tag="qt",
                                          name="qt", bufs=3)
                pq = psum.tile([128, 512], F32, tag="s", name="pq", bufs=2)
                for dt in range(8):
                    nc.tensor.matmul(
                        pq, wq2_sb[:, ct, rc, dt, :], xt_sb[:, rc, dt, :],
                        start=(dt == 0), stop=(dt == 7))
                nc.vector.tensor_scalar_add(
                    qts[ct][:, rc, :], pq, bq2_sb[:, rc, ct:ct + 1])

            def vchunk(ach, ch):
                pv = psum.tile([128, 512], F32, tag="work", name="pv")
                for dt in range(8):
                    nc.tensor.matmul(
                        pv, at_sb[:, dt, ach * 128:(ach + 1) * 128],
                        wv_sb[:, ch, dt, :], start=(dt == 0), stop=(dt == 7))
                pv_v = pv.rearrange("p (hd d) -> p hd d", d=DH)
                bv_v = bv_bc.rearrange(
                    "p (chd hd d) -> p chd hd d", chd=2, d=DH)[:, ch]
                nc.vector.tensor_add(
                    vaug[:, ach, ch * 8:(ch + 1) * 8, 0:DH], pv_v, bv_v)

            def score_span(ct, st, rc, ach):
                # Row-tiled score pair: even head contracts over PE rows
                # 0-63 (tile (0,0)), odd head over rows 64-127 (tile
                # (64,0)); the two matmuls execute concurrently and write
                # separate PSUM banks. One exp covers both heads.
                if ach == 0:
                    st["pts"][rc] = ptpool.tile(
                        [128, 2, 4, 512], BF16, tag="pt", name="pt", bufs=4)
                pt = st["pts"][rc]
                s4 = psum.tile([128, 2, 512], F32, tag="s", name="s4", bufs=2)
                for par in range(2):
                    nc.tensor.matmul(
                        s4[:, par, :],
                        kts[ct][par * 64:(par + 1) * 64,
                                ach * 128:(ach + 1) * 128],
                        qts[ct][par * 64:(par + 1) * 64, rc, :],
                        start=True, stop=True,
                        tile_position=(par * 64, 0))
                nc.scalar.activation(
                    out=pt[:, :, ach, :], in_=s4, func=Exp, scale=SCALE)

            def av_mm(ct, par, st, rc):
                # AV accumulation chain for one row-chunk
                h = 2 * ct + par
                if rc == 0:
                    st[f"pav{par}"] = psum.tile([128, 2, 512], F32,
                                                tag="work", name="pav",
                                                bufs=2)
                pav = st[f"pav{par}"]
                pt = st["pts"][rc]
                for ach in range(4):
                    nc.tensor.matmul(
                        pav[0:DH + 1, rc, :], vaug[:, ach, h, :],
                        pt[:, par, ach, :], start=(ach == 0), stop=(ach == 3))
                if rc == 1:
                    row = par * 64
                    if par == 0:
                        st["praw2"] = tmppool.tile([128, 2, 512], BF16,
                                                   tag="praw", name="praw2")
                    nc.vector.tensor_copy(st["praw2"][row:row + DH, :, :],
                                          pav[0:DH, :, :])
                    nc.vector.tensor_copy(sums4s[ct % 2][row:row + 1, :, :],
                                          pav[DH:DH + 1, :, :])

            def stage_recip(ct, st):
                nc.vector.reciprocal_approx_fast(rcp4, sums4s[ct % 2])
                nc.vector.tensor_copy(rcpbfs[ct % 2], rcp4)

            def stage_norm(ct, st):
                pav1 = st["pav1"]
                for rcn in range(2):
                    nc.tensor.matmul(
                        pav1[:, rcn, :], sel_sb, rcpbfs[ct % 2][:, rcn, :],
                        start=True, stop=True)
                dst = attnT[:, ct, :].rearrange("p (b r) -> p b r", b=2)
                nc.vector.tensor_mul(dst, st["praw2"], pav1)

            # O-proj partials emitted inside the pipeline drain; each chain
            # STARTS with the ones-row bias matmul (adds bo), then
            # accumulates attnT^T @ Wo chunks.
            pouts_head = {}
            hold = {}

            def oproj_start(rti, nh, tag, upto):
                pout = psum.tile([128, 512], F32, tag=tag, name="pout")
                nc.tensor.matmul(pout, ones1,
                                 bo_bc[:, nh * 512:(nh + 1) * 512],
                                 start=True, stop=False)
                for ct2 in range(upto):
                    nc.tensor.matmul(
                        pout, attnT[:, ct2, rti * 128:(rti + 1) * 128],
                        wo_sb[:, ct2, nh * 512:(nh + 1) * 512],
                        start=False, stop=False)
                pouts_head[(rti, nh)] = (pout, upto)

            # ---- pre-pipeline: first two groups' K and Q (rc0) ----
            kproj(0)
            kproj(1)
            qproj_rc(0, 0)
            qproj_rc(1, 0)

            # ---- attention pipeline ----
            sts = {}
            for i in range(10):
                if 2 <= i <= 9:
                    stage_recip(i - 2, sts[i - 2])
                    stage_norm(i - 2, sts[i - 2])
                fills = []
                if i == 0:
                    fills.append(lambda: qproj_rc(0, 1))
                    fills.append(lambda: qproj_rc(1, 1))
                    fills.append(lambda: kproj(2))
                    fills.append(lambda: vchunk(0, 0))
                    fills.append(lambda: vchunk(1, 0))
                if i == 1:
                    fills.append(lambda: vchunk(2, 0))
                    fills.append(lambda: vchunk(3, 0))
                if 2 <= i <= 5:
                    fills.append(lambda i=i: vchunk(i - 2, 1))
                if 1 <= i <= 8:
                    st_p = sts[i - 1]
                    fills.append(
                        lambda st_p=st_p, i=i: av_mm(i - 1, 0, st_p, 0))
                    fills.append(
                        lambda st_p=st_p, i=i: av_mm(i - 1, 0, st_p, 1))
                    fills.append(
                        lambda st_p=st_p, i=i: av_mm(i - 1, 1, st_p, 0))
                    fills.append(
                        lambda st_p=st_p, i=i: av_mm(i - 1, 1, st_p, 1))
                if 1 <= i <= 5:
                    fills.append(lambda i=i: kproj(i + 2))
                if 1 <= i <= 6:
                    fills.append(lambda i=i: qproj_rc(i + 1, 0))
                    fills.append(lambda i=i: qproj_rc(i + 1, 1))
                if i < 8:
                    # 8 row-tiled score spans in 4 blocks of 2 so the PE
                    # only pays a 64<->128 row-mode transition at block
                    # boundaries; fills (128-row-mode matmuls) run between
                    # blocks while the scalar engine's exp stream catches
                    # up.
                    st = sts[i] = {"pts": {}}
                    spans = [(rc, ach) for rc in range(2) for ach in range(4)]
                    nf = len(fills)
                    fi = 0
                    for bi in range(4):
                        for (rc, ach) in spans[2 * bi:2 * bi + 2]:
                            score_span(i, st, rc, ach)
                        upto = nf if bi == 3 else (nf * (bi + 1) + 3) // 4
                        while fi < upto:
                            fills[fi](); fi += 1
                else:
                    fi = 0
                    while fi < len(fills):
                        fills[fi](); fi += 1
                if i == 9:
                    oproj_start(1, 0, "work", 7)
                if i == 8:
                    oproj_start(0, 0, "s", 7)
                    oproj_start(0, 1, "s", 7)

            # ---- output projection; bf16 tiles, evacuation split across
            # Scalar (nh=0) and Vector (nh=1), DMA per 128-row block on
            # alternating queues. ----
            oproj_start(1, 1, "work", 0)
            for rti in range(8):
                out_t = outpool.tile([128, D], BF16, tag="out", name="out_t")
                for nh in range(2):
                    if (rti, nh) in pouts_head:
                        pout, upto = pouts_head[(rti, nh)]
                    else:
                        pout = psum.tile([128, 512], F32, tag="work",
                                         name="pout")
                        nc.tensor.matmul(
                            pout, ones1, bo_bc[:, nh * 512:(nh + 1) * 512],
                            start=True, stop=False)
                        upto = 0
                    for ct2 in range(upto, 8):
                        nc.tensor.matmul(
                            pout, attnT[:, ct2, rti * 128:(rti + 1) * 128],
                            wo_sb[:, ct2, nh * 512:(nh + 1) * 512],
                            start=False, stop=(ct2 == 7))
                    if nh == 0:
                        nc.scalar.copy(out_t[:, 0:512], pout)
                    else:
                        nc.vector.tensor_copy(out_t[:, 512:1024], pout)
                eng = nc.sync if rti % 2 == 0 else nc.scalar
                eng.dma_start(
                    out=out.ap()[rti * 128:(rti + 1) * 128, :], in_=out_t)

    nc.compile()
    return nc


def _swz(a):
    """[1024, cols] -> [128, 8, cols] with row r -> (r % 128, r // 128)."""
    return np.ascontiguousarray(
        a.reshape(8, 128, -1).transpose(1, 0, 2))


def _gmaj(sw):
    """[128, 8dt, 1024] -> [128, 8ct, 8dt, 128] (head-group major)."""
    return np.ascontiguousarray(
        sw.reshape(128, 8, 8, 128).transpose(0, 2, 1, 3))


def _hmaj(sw):
    """[128, 8dt, 1024] -> [128, 2h, 8dt, 512] (column-half major)."""
    return np.ascontiguousarray(
        sw.reshape(128, 8, 2, 512).transpose(0, 2, 1, 3))


def _make_in_maps(x, Wq, bq, Wk, bk, Wv, bv, Wqt, bqt, Wo, bo):
    x = np.asarray(x, dtype=np.float32)
    bf = ml_dtypes.bfloat16

    tobf = lambda w: np.ascontiguousarray(np.asarray(w, np.float32).astype(bf))
    wq_g = _gmaj(_swz(tobf(Wq)))
    wqt_g = _gmaj(_swz(tobf(Wqt)))
    wk_g = _gmaj(_swz(tobf(Wk)))
    wv_g = _hmaj(_swz(tobf(Wv)))
    wo_sw = _swz(tobf(Wo))
    wq2_q0 = np.ascontiguousarray(np.stack([wq_g, wqt_g], axis=2))
    wq2_qx = np.ascontiguousarray(np.stack([wqt_g, wqt_g], axis=2))

    colmaj = lambda v: np.ascontiguousarray(
        np.asarray(v, np.float32).reshape(8, 128).T)
    bq_cm, bqt_cm, bk_cm = map(colmaj, (bq, bqt, bk))
    bq2_q0 = np.ascontiguousarray(np.stack([bq_cm, bqt_cm], axis=1))
    bq2_qx = np.ascontiguousarray(np.stack([bqt_cm, bqt_cm], axis=1))
    bv_bc = np.ascontiguousarray(
        np.broadcast_to(np.asarray(bv, np.float32).astype(bf), (128, D)))
    bo_bc = np.ascontiguousarray(
        np.broadcast_to(np.asarray(bo, np.float32).astype(bf), (128, D)))

    at_sw = [_swz(x[b, :A, :].T.astype(bf)) for b in range(B)]
    in_maps = []
    for c in range(NCORES):
        b, q = divmod(c, 4)
        rows = x[b, q * RPC:(q + 1) * RPC, :]
        in_maps.append({
            "xt": _hmaj(_swz(rows.T.astype(bf))),
            "at": at_sw[b],
            "wq2": wq2_q0 if q == 0 else wq2_qx,
            "wk": wk_g, "wv": wv_g, "wo": wo_sw,
            "bq2": bq2_q0 if q == 0 else bq2_qx,
            "bk": bk_cm, "bv": bv_bc, "bo": bo_bc,
        })
    return in_maps


def kernel(x, Wq, bq, Wk, bk, Wv, bv, Wqt, bqt, Wo, bo, num_anchor_tokens):
    assert int(num_anchor_tokens) == A
    if "nc" not in _CACHE:
        _CACHE["nc"] = _build()
    nc = _CACHE["nc"]

    in_maps = _make_in_maps(x, Wq, bq, Wk, bk, Wv, bv, Wqt, bqt, Wo, bo)
    res = bass_utils.run_bass_kernel_spmd(
        nc, in_maps, core_ids=list(range(NCORES)))
    out = np.empty((B, S, D), np.float32)
    for c in range(NCORES):
        b, q = divmod(c, 4)
        out[b, q * RPC:(q + 1) * RPC, :] = np.asarray(
            res.results[c]["out"], np.float32)
    return out


# revision 26
# speedup vs baseline: 1.0025x; 1.0025x over previous
"""AnchorAttention distributed Bass kernel for 8 TRN2 NeuronCores.

Reference computation (B=2, S=4096, D=1024, H=16, Dh=64, A=512):
  anchors = x[:, :A];  queries = x[:, A:]
  anchor_q/k/v = split_heads(anchors @ Wq/Wk/Wv + b)
  query_q      = split_heads(queries @ Wqt + bqt)
  combined_q   = concat([anchor_q, query_q], axis=2)       # [B,H,S,Dh]
  out  = softmax(combined_q @ anchor_k^T / sqrt(Dh)) @ anchor_v
  out  = merge_heads(out) @ Wo + bo

Sharding: the B*S = 8192 token rows are split into 8 chunks of 1024 rows
(core c -> batch c//4, rows (c%4)*1024 ...). Each core duplicates its
batch's anchor K/V projections, computes Q for its own rows (Wq for the
anchor-region rows, Wqt for query rows), attention over the 512 anchors
for all 16 heads, and the output projection for its rows. The output is a
pure concatenation: no collectives.

Layout: everything is kept transposed ([feature, row]) so each matmul
contracts over the partition dim with zero on-chip transposes. Host
pre-transposes/pre-casts to bf16 (f32 accumulation in PSUM). Weights are
laid out head-group(ct)-major so each group's projection only depends on
a small prefix of the DMA stream; K/V projections are woven per-group
into the attention pipeline so compute starts as soon as the first
chunks land. A short burst of dummy matmuls at t=0 warms the PE clock
gate (HAM) before real work arrives.

Scores contract 64-deep per head, so each (row-chunk, anchor-chunk)
score pair runs as two concurrent PE row-tiles ((0,0) even head /
(64,0) odd head) writing separate PSUM banks -- 2x score throughput and
no zero-padded Q slabs. Score spans are emitted in blocks of two so the
PE only pays a row-mode transition at block boundaries.

Softmax row-sums come free via an extra all-ones column appended to V
(no max-subtraction is needed; scores are ~N(0,1)). 1/sums: sums rows
are copied from PSUM to fixed parity SBUF tiles (custom-DVE reciprocal
cannot read PSUM), reciprocal'd + cast on Vector, partition-broadcast by
a [65,128] 0/1-selector matmul, and multiplied into the attn^T slab.

Output projection: bo is folded into the PE accumulation via a
ones-first-row stationary matmul against a broadcast bo tile, so the
PSUM evacuation is a plain copy, split between the Scalar and Vector
engines, cast to bf16, and DMA'd out per 128-row block on alternating
queues so the drain pipelines with the O-proj matmul stream.
"""

import numpy as np
import ml_dtypes

import concourse.bass as bass
import concourse.tile as tile
from concourse import bacc, mybir
from concourse import bass_utils

BF16 = mybir.dt.bfloat16
F32 = mybir.dt.float32
B, S, D = 2, 4096, 1024
H, DH = 16, 64
A = 512                  # num_anchor_tokens (asserted at runtime)
RPC = 1024               # rows per core
NCORES = 8
SCALE = 1.0 / np.sqrt(float(DH))

_CACHE = {}


def _build():
    """Build + compile the per-core Bass graph (identical on all cores)."""
    nc = bacc.Bacc("TRN2", target_bir_lowering=False, debug=False)

    # [feat%128, rc, dt, row]: row chunks of x^T, dt = feat//128
    xt = nc.dram_tensor("xt", [128, 2, 8, 512], BF16, kind="ExternalInput")
    at = nc.dram_tensor("at", [128, 8, A], BF16, kind="ExternalInput")
    # Q weights, group-major: [in%128, ct, rc, dt, out-slice]
    wq2 = nc.dram_tensor("wq2", [128, 8, 2, 8, 128], BF16,
                         kind="ExternalInput")
    wk = nc.dram_tensor("wk", [128, 8, 8, 128], BF16, kind="ExternalInput")
    wv = nc.dram_tensor("wv", [128, 2, 8, 512], BF16, kind="ExternalInput")
    wo = nc.dram_tensor("wo", [128, 8, D], BF16, kind="ExternalInput")
    bq2 = nc.dram_tensor("bq2", [128, 2, 8], F32, kind="ExternalInput")
    bk = nc.dram_tensor("bk", [128, 8], F32, kind="ExternalInput")
    bv = nc.dram_tensor("bv", [128, D], BF16, kind="ExternalInput")  # bcast
    bo = nc.dram_tensor("bo", [128, D], BF16, kind="ExternalInput")  # bcast
    out = nc.dram_tensor("out", [RPC, D], BF16, kind="ExternalOutput")

    Exp = mybir.ActivationFunctionType.Exp
    Ident = mybir.ActivationFunctionType.Identity

    from contextlib import ExitStack

    with tile.TileContext(nc) as tc:
        with tc.tile_pool(name="wpool", bufs=1) as wpool, \
             tc.tile_pool(name="cpool", bufs=1) as cpool, \
             tc.tile_pool(name="kvpool", bufs=1) as kvpool, \
             tc.tile_pool(name="ptpool", bufs=4) as ptpool, \
             tc.tile_pool(name="tmppool", bufs=2) as tmppool, \
             tc.tile_pool(name="outpool", bufs=2) as outpool, \
             tc.tile_pool(name="attnpool", bufs=1) as attnpool, \
             tc.tile_pool(name="psum", bufs=2, space="PSUM") as psum:


            # ---- constants (GpSimd so the DMA queues stay clear) ----
            sel_sb = cpool.tile([65, 128], BF16, name="sel_sb")
            nc.gpsimd.memset(sel_sb, 0.0)
            nc.gpsimd.memset(sel_sb[0:1, 0:64], 1.0)
            nc.gpsimd.memset(sel_sb[64:65, 64:128], 1.0)
            # ones-first-row stationary for the O-proj bias accumulation
            ones1 = cpool.tile([128, 128], BF16, name="ones1")
            nc.gpsimd.memset(ones1, 0.0)
            nc.gpsimd.memset(ones1[0:1, :], 1.0)
            wtile = cpool.tile([65, 128], BF16, name="wtile")
            nc.gpsimd.memset(wtile, 0.0)

            # ---- input DMAs, deadline-ordered, chunked across the three
            # HWDGE dispatch queues (sync / scalar / gpsimd). ----
            at_sb = wpool.tile([128, 8, A], BF16, name="at_sb")
            wk_sb = wpool.tile([128, 8, 8, 128], BF16, name="wk_sb")
            xt_sb = wpool.tile([128, 2, 8, 512], BF16, name="xt_sb")
            wv_sb = wpool.tile([128, 2, 8, 512], BF16, name="wv_sb")
            wq2_sb = wpool.tile([128, 8, 2, 8, 128], BF16, name="wq2_sb")
            wo_sb = wpool.tile([128, 8, D], BF16, name="wo_sb")

            nc.sync.dma_start(out=at_sb[:, 0:4, :], in_=at.ap()[:, 0:4, :])
            nc.sync.dma_start(out=wk_sb[:, 0:2], in_=wk.ap()[:, 0:2])
            nc.sync.dma_start(out=xt_sb[:, 0], in_=xt.ap()[:, 0])
            nc.sync.dma_start(out=wk_sb[:, 2:4], in_=wk.ap()[:, 2:4])
            nc.sync.dma_start(out=wv_sb[:, 0], in_=wv.ap()[:, 0])
            nc.sync.dma_start(out=wk_sb[:, 4:8], in_=wk.ap()[:, 4:8])
            nc.sync.dma_start(out=wo_sb[:, 0:4, :], in_=wo.ap()[:, 0:4, :])
            nc.sync.dma_start(out=wo_sb[:, 4:8, :], in_=wo.ap()[:, 4:8, :])

            nc.scalar.dma_start(out=at_sb[:, 4:8, :], in_=at.ap()[:, 4:8, :])
            nc.scalar.dma_start(out=wq2_sb[:, 0:2], in_=wq2.ap()[:, 0:2])
            nc.scalar.dma_start(out=xt_sb[:, 1], in_=xt.ap()[:, 1])
            nc.scalar.dma_start(out=wq2_sb[:, 2:4], in_=wq2.ap()[:, 2:4])
            nc.scalar.dma_start(out=wq2_sb[:, 4:6], in_=wq2.ap()[:, 4:6])
            nc.scalar.dma_start(out=wv_sb[:, 1], in_=wv.ap()[:, 1])


            bq2_sb = cpool.tile([128, 2, 8], F32, name="bq2_sb")
            bk_sb = cpool.tile([128, 8], F32, name="bk_sb")
            bv_bc = cpool.tile([128, D], BF16, name="bv_bc")
            bo_bc = cpool.tile([128, D], BF16, name="bo_bc")
            nc.gpsimd.dma_start(out=bq2_sb, in_=bq2.ap())
            nc.gpsimd.dma_start(out=bk_sb, in_=bk.ap())
            nc.gpsimd.dma_start(out=bv_bc, in_=bv.ap())
            nc.gpsimd.dma_start(out=bo_bc, in_=bo.ap())
            nc.gpsimd.dma_start(out=wq2_sb[:, 6:8], in_=wq2.ap()[:, 6:8])

            # ---- PE warmup: dep-free dummy matmuls fill the DMA head so
            # the HAM clock gate is at full rate when real work arrives ----
            pwarm = psum.tile([128, 128], F32, tag="s", name="pwarm", bufs=2)
            for _ in range(72):
                nc.tensor.matmul(pwarm, sel_sb, wtile, start=True, stop=True)

            # fixed parity tiles for the 1/sums chain; rows other than 0/64
            # hold 1.0 forever so the reciprocal/cast stay finite
            sums4s, rcpbfs = [], []
            rcp4 = cpool.tile([65, 2, 512], F32, name="rcp4")
            for eo in range(2):
                s4p = cpool.tile([65, 2, 512], F32, name=f"sums4_{eo}")
                nc.vector.memset(s4p, 1.0)
                sums4s.append(s4p)
                rcpbfs.append(cpool.tile([65, 2, 512], BF16,
                                         name=f"rcpbf_{eo}"))

            # V slab: [128(a%128), ach, head, 65]; cols 0-63 = V head slice,
            # col 64 = ones (supplies softmax row-sums during AV).
            vaug = kvpool.tile([128, 4, H, DH + 1], BF16, name="vaug")
            nc.vector.memset(vaug, 1.0)

            kts, qts = {}, {}
            attnT = attnpool.tile([128, 8, RPC], BF16, name="attnT")

            # ---- per-group projections (woven into the pipeline) ----
            def kproj(ct):
                kts[ct] = kvpool.tile([128, A], BF16, tag="kt",
                                      name="kt", bufs=3)
                pk = psum.tile([128, A], F32, tag="work", name="pk")
                for dt in range(8):
                    nc.tensor.matmul(
                        pk, wk_sb[:, ct, dt, :], at_sb[:, dt, :],
                        start=(dt == 0), stop=(dt == 7))
                nc.scalar.activation(out=kts[ct], in_=pk,
                                     func=Ident, bias=bk_sb[:, ct:ct + 1])

            def qproj_rc(ct, rc):
                if rc == 0:
                    qts[ct] = kvpool.tile([128, 2, 512], BF16, tag="qt",
                                          name="qt", bufs=3)
                pq = psum.tile([128, 512], F32, tag="s", name="pq", bufs=2)
                for dt in range(8):
                    nc.tensor.matmul(
                        pq, wq2_sb[:, ct, rc, dt, :], xt_sb[:, rc, dt, :],
                        start=(dt == 0), stop=(dt == 7))
                nc.vector.tensor_scalar_add(
                    qts[ct][:, rc, :], pq, bq2_sb[:, rc, ct:ct + 1])

            def vchunk(ach, ch):
                pv = psum.tile([128, 512], F32, tag="work", name="pv")
                for dt in range(8):
                    nc.tensor.matmul(
                        pv, at_sb[:, dt, ach * 128:(ach + 1) * 128],
                        wv_sb[:, ch, dt, :], start=(dt == 0), stop=(dt == 7))
                pv_v = pv.rearrange("p (hd d) -> p hd d", d=DH)
                bv_v = bv_bc.rearrange(
                    "p (chd hd d) -> p chd hd d", chd=2, d=DH)[:, ch]
                nc.vector.tensor_add(
                    vaug[:, ach, ch * 8:(ch + 1) * 8, 0:DH], pv_v, bv_v)

            def score_span(ct, st, rc, ach):
                # Row-tiled score pair: even head contracts over PE rows
                # 0-63 (tile (0,0)), odd head over rows 64-127 (tile
                # (64,0)); the two matmuls execute concurrently and write
                # separate PSUM banks. One exp covers both heads.
                if ach == 0:
                    st["pts"][rc] = ptpool.tile(
                        [128, 2, 4, 512], BF16, tag="pt", name="pt", bufs=4)
                pt = st["pts"][rc]
                s4 = psum.tile([128, 2, 512], F32, tag="s", name="s4", bufs=2)
                for par in range(2):
                    nc.tensor.matmul(
                        s4[:, par, :],
                        kts[ct][par * 64:(par + 1) * 64,
                                ach * 128:(ach + 1) * 128],
                        qts[ct][par * 64:(par + 1) * 64, rc, :],
                        start=True, stop=True,
                        tile_position=(par * 64, 0))
                nc.scalar.activation(
                    out=pt[:, :, ach, :], in_=s4, func=Exp, scale=SCALE)

            def av_mm(ct, par, st, rc):
                # AV accumulation chain for one row-chunk
                h = 2 * ct + par
                if rc == 0:
                    st[f"pav{par}"] = psum.tile([128, 2, 512], F32,
                                                tag="work", name="pav",
                                                bufs=2)
                pav = st[f"pav{par}"]
                pt = st["pts"][rc]
                for ach in range(4):
                    nc.tensor.matmul(
                        pav[0:DH + 1, rc, :], vaug[:, ach, h, :],
                        pt[:, par, ach, :], start=(ach == 0), stop=(ach == 3))
                if rc == 1:
                    row = par * 64
                    if par == 0:
                        st["praw2"] = tmppool.tile([128, 2, 512], BF16,
                                                   tag="praw", name="praw2")
                    nc.vector.tensor_copy(st["praw2"][row:row + DH, :, :],
                                          pav[0:DH, :, :])
                    nc.vector.tensor_copy(sums4s[ct % 2][row:row + 1, :, :],
                                          pav[DH:DH + 1, :, :])

            def stage_recip(ct, st):
                nc.vector.reciprocal_approx_fast(rcp4, sums4s[ct % 2])
                nc.vector.tensor_copy(rcpbfs[ct % 2], rcp4)

            def stage_norm(ct, st):
                pav1 = st["pav1"]
                for rcn in range(2):
                    nc.tensor.matmul(
                        pav1[:, rcn, :], sel_sb, rcpbfs[ct % 2][:, rcn, :],
                        start=True, stop=True)
                dst = attnT[:, ct, :].rearrange("p (b r) -> p b r", b=2)
                nc.vector.tensor_mul(dst, st["praw2"], pav1)

            # O-proj partials emitted inside the pipeline drain; each chain
            # STARTS with the ones-row bias matmul (adds bo), then
            # accumulates attnT^T @ Wo chunks.
            pouts_head = {}
            hold = {}

            def oproj_start(rti, nh, tag, upto):
                pout = psum.tile([128, 512], F32, tag=tag, name="pout")
                nc.tensor.matmul(pout, ones1,
                                 bo_bc[:, nh * 512:(nh + 1) * 512],
                                 start=True, stop=False)
                for ct2 in range(upto):
                    nc.tensor.matmul(
                        pout, attnT[:, ct2, rti * 128:(rti + 1) * 128],
                        wo_sb[:, ct2, nh * 512:(nh + 1) * 512],
                        start=False, stop=False)
                pouts_head[(rti, nh)] = (pout, upto)

            # ---- pre-pipeline: first two groups' K and Q (rc0) ----
            kproj(0)
            kproj(1)
            qproj_rc(0, 0)
            qproj_rc(1, 0)

            # ---- attention pipeline ----
            sts = {}
            for i in range(10):
                if 2 <= i <= 9:
                    stage_recip(i - 2, sts[i - 2])
                    stage_norm(i - 2, sts[i - 2])
                fills = []
                if i == 0:
                    fills.append(lambda: qproj_rc(0, 1))
                    fills.append(lambda: qproj_rc(1, 1))
                    fills.append(lambda: kproj(2))
                    fills.append(lambda: vchunk(0, 0))
                    fills.append(lambda: vchunk(1, 0))
                if i == 1:
                    fills.append(lambda: vchunk(2, 0))
                    fills.append(lambda: vchunk(3, 0))
                if 2 <= i <= 5:
                    fills.append(lambda i=i: vchunk(i - 2, 1))
                if 1 <= i <= 8:
                    st_p = sts[i - 1]
                    fills.append(
                        lambda st_p=st_p, i=i: av_mm(i - 1, 0, st_p, 0))
                    fills.append(
                        lambda st_p=st_p, i=i: av_mm(i - 1, 0, st_p, 1))
                    fills.append(
                        lambda st_p=st_p, i=i: av_mm(i - 1, 1, st_p, 0))
                    fills.append(
                        lambda st_p=st_p, i=i: av_mm(i - 1, 1, st_p, 1))
                if 1 <= i <= 5:
                    fills.append(lambda i=i: kproj(i + 2))
                if 1 <= i <= 6:
                    fills.append(lambda i=i: qproj_rc(i + 1, 0))
                    fills.append(lambda i=i: qproj_rc(i + 1, 1))
                if i < 8:
                    # 8 row-tiled score spans in 4 blocks of 2 so the PE
                    # only pays a 64<->128 row-mode transition at block
                    # boundaries; fills (128-row-mode matmuls) run between
                    # blocks while the scalar engine's exp stream catches
                    # up.
                    st = sts[i] = {"pts": {}}
                    spans = [(rc, ach) for rc in range(2) for ach in range(4)]
                    nf = len(fills)
                    fi = 0
                    for bi in range(4):
                        for (rc, ach) in spans[2 * bi:2 * bi + 2]:
                            score_span(i, st, rc, ach)
                        upto = nf if bi == 3 else (nf * (bi + 1) + 3) // 4
                        while fi < upto:
                            fills[fi](); fi += 1
                else:
                    fi = 0
                    while fi < len(fills):
                        fills[fi](); fi += 1
                if i == 9:
                    oproj_start(1, 0, "work", 7)
                if i == 8:
                    oproj_start(0, 0, "s", 7)
                    oproj_start(0, 1, "s", 7)

            # ---- output projection; bf16 tiles, evacuation split across
            # Scalar (nh=0) and Vector (nh=1), DMA per 128-row block on
            # alternating queues. ----
            oproj_start(1, 1, "work", 0)
            for rti in range(8):
                out_t = outpool.tile([128, D], BF16, tag="out", name="out_t")
                for nh in range(2):
                    if (rti, nh) in pouts_head:
                        pout, upto = pouts_head[(rti, nh)]
                    else:
                        pout = psum.tile([128, 512], F32, tag="work",
                                         name="pout")
                        nc.tensor.matmul(
                            pout, ones1, bo_bc[:, nh * 512:(nh + 1) * 512],
                            start=True, stop=False)
                        upto = 0
                    for ct2 in range(upto, 8):
                        nc.tensor.matmul(
                            pout, attnT[:, ct2, rti * 128:(rti + 1) * 128],
                            wo_sb[:, ct2, nh * 512:(nh + 1) * 512],
                            start=False, stop=(ct2 == 7))
                    if nh == 0:
                        nc.scalar.copy(out_t[:, 0:512], pout)
                    else:
                        nc.vector.tensor_copy(out_t[:, 512:1024], pout)
                eng = nc.sync if rti % 2 == 0 else nc.scalar
                eng.dma_start(
                    out=out.ap()[rti * 128:(rti + 1) * 128, :], in_=out_t)

    nc.compile()
    return nc


def _swz(a):
    """[1024, cols] -> [128, 8, cols] with row r -> (r % 128, r // 128)."""
    return np.ascontiguousarray(
        a.reshape(8, 128, -1).transpose(1, 0, 2))


def _gmaj(sw):
    """[128, 8dt, 1024] -> [128, 8ct, 8dt, 128] (head-group major)."""
    return np.ascontiguousarray(
        sw.reshape(128, 8, 8, 128).transpose(0, 2, 1, 3))


def _hmaj(sw):
    """[128, 8dt, 1024] -> [128, 2h, 8dt, 512] (column-half major)."""
    return np.ascontiguousarray(
        sw.reshape(128, 8, 2, 512).transpose(0, 2, 1, 3))


def _make_in_maps(x, Wq, bq, Wk, bk, Wv, bv, Wqt, bqt, Wo, bo):
    x = np.asarray(x, dtype=np.float32)
    bf = ml_dtypes.bfloat16

    tobf = lambda w: np.ascontiguousarray(np.asarray(w, np.float32).astype(bf))
    wq_g = _gmaj(_swz(tobf(Wq)))
    wqt_g = _gmaj(_swz(tobf(Wqt)))
    wk_g = _gmaj(_swz(tobf(Wk)))
    wv_g = _hmaj(_swz(tobf(Wv)))
    wo_sw = _swz(tobf(Wo))
    wq2_q0 = np.ascontiguousarray(np.stack([wq_g, wqt_g], axis=2))
    wq2_qx = np.ascontiguousarray(np.stack([wqt_g, wqt_g], axis=2))

    colmaj = lambda v: np.ascontiguousarray(
        np.asarray(v, np.float32).reshape(8, 128).T)
    bq_cm, bqt_cm, bk_cm = map(colmaj, (bq, bqt, bk))
    bq2_q0 = np.ascontiguousarray(np.stack([bq_cm, bqt_cm], axis=1))
    bq2_qx = np.ascontiguousarray(np.stack([bqt_cm, bqt_cm], axis=1))
    bv_bc = np.ascontiguousarray(
        np.broadcast_to(np.asarray(bv, np.float32).astype(bf), (128, D)))
    bo_bc = np.ascontiguousarray(
        np.broadcast_to(np.asarray(bo, np.float32).astype(bf), (128, D)))

    at_sw = [_swz(x[b, :A, :].T.astype(bf)) for b in range(B)]
    in_maps = []
    for c in range(NCORES):
        b, q = divmod(c, 4)
        rows = x[b, q * RPC:(q + 1) * RPC, :]
        in_maps.append({
            "xt": _hmaj(_swz(rows.T.astype(bf))),
            "at": at_sw[b],
            "wq2": wq2_q0 if q == 0 else wq2_qx,
            "wk": wk_g, "wv": wv_g, "wo": wo_sw,
            "bq2": bq2_q0 if q == 0 else bq2_qx,
            "bk": bk_cm, "bv": bv_bc, "bo": bo_bc,
        })
    return in_maps


def kernel(x, Wq, bq, Wk, bk, Wv, bv, Wqt, bqt, Wo, bo, num_anchor_tokens):
    assert int(num_anchor_tokens) == A
    if "nc" not in _CACHE:
        _CACHE["nc"] = _build()
    nc = _CACHE["nc"]

    in_maps = _make_in_maps(x, Wq, bq, Wk, bk, Wv, bv, Wqt, bqt, Wo, bo)
    res = bass_utils.run_bass_kernel_spmd(
        nc, in_maps, core_ids=list(range(NCORES)))
    out = np.empty((B, S, D), np.float32)
    for c in range(NCORES):
        b, q = divmod(c, 4)
        out[b, q * RPC:(q + 1) * RPC, :] = np.asarray(
            res.results[c]["out"], np.float32)
    return out
